# revision 14
# baseline (speedup 1.0000x reference)
"""Trainium2 Bass kernel for nn_AttentionNetwork (ragged path attention).

Data-parallel over 8 NeuronCores: 512 paths per core, dealt round-robin by
global length-sorted rank so all 8 cores see near-identical sorted length
profiles (tight SPMD packing). Paths are packed into variable-width blocks
(bp paths x cap node-slots, cap a multiple of 4, bp*cap <= 512).

Stage 1 per block, all-bf16 on the PE (78.6 TF/s), in transposed layout:
stationary = X row-chunks, moving = W1' where W1' columns are reordered by
sign(w2) and scaled by |w2| (relu(h)@w2 == sum relu(h*|w2|)_pos - sum
relu(h*|w2|)_neg since |w2|>=0 commutes with relu). PSUM holds h'
[rows<=128, H]; relu + the w2 contraction then FUSE into two scalar-engine
activations via accum_out, so no score matmuls and no rh tensor exist at
all. Scores come out one-per-partition [128, nchunk]; exp is applied there
(tiny), a PE transpose + DMA flattens the weights to [1, rows], and gpsimd
broadcasts them into channel KC of the xw tile. Padded node slots are
zeroed host-side (score exactly relu(b1)@w2 = const c0); the softmax
denominator is corrected by subtracting npad*exp(c0) instead of masking.
The softmax-weighted node sum reduces channels [x*w (4), w (1)] together
with log2 halving tensor_adds in bf16 (DVE 2x mode) plus a final 1x reduce.

Stage 2 (f32r): path-level attention over [128, KC, PS] path features;
host combines the 8 exp-weighted partials (softmax over paths is
permutation-invariant, so the dealt order needs no undoing).
"""

import sys

if "/opt/trn_rl_repo" not in sys.path:
    sys.path.insert(0, "/opt/trn_rl_repo")

from contextlib import ExitStack

import ml_dtypes
import numpy as np

import concourse.bass as bass  # noqa: F401
import concourse.masks as masks
import concourse.mybir as mybir
import concourse.tile as tile
from concourse import bacc, bass_utils

P, LMAX, D, H = 4096, 64, 512, 512
NCORES = 8
PS = P // NCORES          # paths per core
KC = D // 128             # contraction chunks
ROWS_TARGET = 512         # max rows (bp*cap) per block

f32 = mybir.dt.float32
f32r = mybir.dt.float32r
bf16 = mybir.dt.bfloat16
AF = mybir.ActivationFunctionType
ALU = mybir.AluOpType
AX = mybir.AxisListType

LAST_RESULT = None
_PROG_CACHE = {}
_TRACE_KW = {}


def _make_blocks(len_max):
    """Greedy pack sorted-desc lengths into (bp, cap) blocks, bp*cap<=512."""
    blocks = []
    i = 0
    while i < PS:
        cap = int(len_max[i])
        if cap > 4:
            cap = (cap + 3) // 4 * 4   # mult-of-4 caps: aligned halving adds
        else:
            cap += cap & 1             # even free dims for the PE
        bp = min(ROWS_TARGET // cap, PS - i)
        blocks.append((bp, cap))
        i += bp
    return tuple(blocks)


def _build_program(blocks, npos, has_b1):
    """blocks: tuple of (bp, cap); one block = bp paths x cap node slots."""
    nb = len(blocks)
    rows_list = [bp * cap for bp, cap in blocks]
    tot_rows = sum(rows_list)
    NCH = KC + 1              # 4 x*w channels + 1 weight channel

    nc = bacc.Bacc("TRN2", target_bir_lowering=False, debug=False, num_devices=NCORES)

    xb = nc.dram_tensor("xb", [KC * 128 * tot_rows], bf16, kind="ExternalInput")
    npad = nc.dram_tensor("npad", [128, PS], f32, kind="ExternalInput")
    w1 = nc.dram_tensor("w1", [KC, 128, H], bf16, kind="ExternalInput")
    b1r = nc.dram_tensor("b1r", [1, H], bf16, kind="ExternalInput")
    aw1 = nc.dram_tensor("aw1", [KC, 128, H], f32r, kind="ExternalInput")
    ab1 = nc.dram_tensor("ab1", [128, KC], f32, kind="ExternalInput")
    aw2 = nc.dram_tensor("aw2", [128, KC], f32r, kind="ExternalInput")
    one1_bf = nc.dram_tensor("one1_bf", [1, 128], bf16, kind="ExternalInput")
    out_part = nc.dram_tensor("out_part", [128, KC], f32, kind="ExternalOutput")
    out_stats = nc.dram_tensor("out_stats", [1, 2], f32, kind="ExternalOutput")

    with ExitStack() as ctx:
        tc = ctx.enter_context(tile.TileContext(nc))
        const = ctx.enter_context(tc.tile_pool(name="const", bufs=1))
        xpool = ctx.enter_context(tc.tile_pool(name="x", bufs=4))
        xwpool = ctx.enter_context(tc.tile_pool(name="xw", bufs=3))
        hpool = ctx.enter_context(tc.tile_pool(name="h", bufs=3))
        vpool = ctx.enter_context(tc.tile_pool(name="v", bufs=2))
        spool = ctx.enter_context(tc.tile_pool(name="s", bufs=3))
        ph_pool = ctx.enter_context(tc.tile_pool(name="ph", bufs=5, space="PSUM"))
        ps_pool = ctx.enter_context(tc.tile_pool(name="ps", bufs=2, space="PSUM"))

        t_w1 = const.tile([128, KC, H], bf16)
        nc.sync.dma_start(t_w1[:], w1.ap().rearrange("k d h -> d k h"))
        t_npad = const.tile([128, PS], f32)
        nc.sync.dma_start(t_npad[:], npad.ap())
        t_one1 = const.tile([1, 128], bf16)
        nc.sync.dma_start(t_one1[:], one1_bf.ap())
        t_b1r = const.tile([1, H], bf16)
        nc.sync.dma_start(t_b1r[:], b1r.ap())
        t_ident = const.tile([128, 128], bf16)
        masks.make_identity(nc, t_ident[:])
        # ACT table prefetch: force the exp_and_others load before data arrives
        t_warm = const.tile([1, 1], f32)
        nc.scalar.activation(t_warm[:], t_one1[:, 0:1], AF.Exp)
        t_aw1 = const.tile([128, KC, H], f32r)
        t_ab1 = const.tile([128, KC], f32)
        t_aw2 = const.tile([128, KC], f32r)

        pfT = const.tile([128, KC, PS], f32r)  # normalized path features

        x_offs = [0] * nb
        p_offs = [0] * nb
        acc_x = acc_p = 0
        for i in range(nb):
            x_offs[i], p_offs[i] = acc_x, acc_p
            acc_x += KC * 128 * rows_list[i]
            acc_p += blocks[i][0]
        assert acc_p == PS

        emit_order = [nb - 1] + list(range(nb - 1))
        for ei, b in enumerate(emit_order):
            bp, cap = blocks[b]
            rows = rows_list[b]
            x_off, p_off = x_offs[b], p_offs[b]
            ncb = (rows + 127) // 128

            x_b = xpool.tile([128, KC, rows], bf16, tag="xb", name=f"xb_{b}")
            nc.sync.dma_start(
                x_b[:],
                xb.ap()[x_off : x_off + KC * 128 * rows].rearrange(
                    "(k d r) -> d k r", k=KC, d=128
                ),
            )

            accs = spool.tile([128, 2 * ncb], f32, tag="accs", name=f"accs_{b}")
            accA = accs[:, 0:ncb]
            accB = accs[:, ncb : 2 * ncb]
            for ci in range(ncb):
                r0 = ci * 128
                rc = min(128, rows - r0)
                ph = ph_pool.tile([128, H], f32, tag="h", name=f"ph_{b}_{ci}")
                for k in range(KC):
                    nc.tensor.matmul(
                        ph[0:rc, :],
                        x_b[:, k, r0 : r0 + rc],
                        t_w1[:, k, :],
                        start=(k == 0),
                        stop=(k == KC - 1) and not has_b1,
                    )
                if has_b1:
                    nc.tensor.matmul(
                        ph[0:rc, :], t_one1[:, 0:rc], t_b1r[:],
                        start=False, stop=True,
                    )
                dum = hpool.tile([128, H], bf16, tag="dum", name=f"dum_{b}_{ci}")
                if npos > 0:
                    nc.scalar.activation(
                        dum[0:rc, 0:npos], ph[0:rc, 0:npos], AF.Relu,
                        accum_out=accA[0:rc, ci : ci + 1],
                    )
                else:
                    nc.vector.memset(accA[:, ci : ci + 1], 0.0)
                if npos < H:
                    nc.scalar.activation(
                        dum[0:rc, npos:H], ph[0:rc, npos:H], AF.Relu,
                        accum_out=accB[0:rc, ci : ci + 1],
                    )
                else:
                    nc.vector.memset(accB[:, ci : ci + 1], 0.0)

            score = spool.tile([128, ncb], f32, tag="score", name=f"sc_{b}")
            nc.vector.tensor_sub(score[:], accA, accB)
            wcol = spool.tile([128, ncb], bf16, tag="wcol", name=f"wc_{b}")
            nc.scalar.activation(wcol[:], score[:], AF.Exp)
            wrow = ps_pool.tile([128, 128], bf16, tag="tr", name=f"wr_{b}")
            nc.tensor.transpose(wrow[0:ncb, :], wcol[:], t_ident[:])
            srow = spool.tile([128, 128], bf16, tag="srow", name=f"sr_{b}")
            nc.scalar.copy(srow[0:ncb, :], wrow[0:ncb, :])
            erow = spool.tile([1, ncb * 128], bf16, tag="erow", name=f"er_{b}")
            nc.scalar.dma_start(
                erow[:].rearrange("o (c p) -> o c p", p=128), srow[0:ncb, :]
            )

            # xwt channels 0..KC-1 = x*w ; channel KC = w (broadcast weights)
            xwt = xwpool.tile([128, NCH, rows], bf16, tag="xw", name=f"xw_{b}")
            nc.gpsimd.partition_broadcast(xwt[:, KC, :], erow[:, 0:rows])
            for k in range(KC):
                nc.vector.tensor_mul(xwt[:, k, :], x_b[:, k, :], xwt[:, KC, :])

            # segmented sum over cap: bf16 halving adds (2x DVE) + final reduce
            nseg = NCH * bp
            cur_ap = xwt[:].rearrange("p f (s l) -> p (f s) l", l=cap)
            cc = cap
            lvl = 0
            while cc % 2 == 0 and cc > 2:
                half = cc // 2
                nxt = vpool.tile(
                    [128, nseg * half], bf16, tag=f"hv{lvl}", name=f"hv{lvl}_{b}"
                )
                nxt_ap = nxt[:].rearrange("p (f l) -> p f l", l=half)
                nc.vector.tensor_add(
                    nxt_ap, cur_ap[:, :, 0:half], cur_ap[:, :, half:cc]
                )
                cur_ap = nxt_ap
                cc = half
                lvl += 1
            praw = spool.tile([128, NCH * bp], f32, tag="praw", name=f"praw_{b}")
            praw_seg = praw[:].rearrange("p (f s) -> p f s", f=NCH)
            nc.vector.reduce_sum(praw[:], cur_ap, axis=AX.X)

            wcor = spool.tile([128, bp], f32, tag="wcor", name=f"wcor_{b}")
            nc.vector.tensor_sub(
                wcor[:], praw_seg[:, KC, :], t_npad[:, p_off : p_off + bp]
            )
            winv = spool.tile([128, bp], f32, tag="winv", name=f"winv_{b}")
            nc.vector.reciprocal(winv[:], wcor[:])
            winv_bc = winv[:].rearrange("p (x s) -> p x s", x=1).to_broadcast(
                [128, KC, bp]
            )
            nc.vector.tensor_mul(
                pfT[:, :, p_off : p_off + bp], praw_seg[:, 0:KC, :], winv_bc
            )

            if ei == 0:
                nc.sync.dma_start(t_aw1[:], aw1.ap().rearrange("k d h -> d k h"))
                nc.sync.dma_start(t_ab1[:], ab1.ap())
                nc.sync.dma_start(t_aw2[:], aw2.ap())

        # ---- stage 2: path-level attention (f32r matmuls) ----
        pfr = pfT[:]
        rh2_list = []
        for j in range(KC):
            ph2 = ph_pool.tile([128, PS], f32, tag="h")
            for k in range(KC):
                nc.tensor.matmul(
                    ph2[:],
                    t_aw1[:, k, 128 * j : 128 * (j + 1)],
                    pfr[:, k, :],
                    start=(k == 0),
                    stop=(k == KC - 1),
                )
            rh2 = hpool.tile([128, PS], f32r, tag=f"rh2{j}")
            nc.scalar.activation(rh2[:], ph2[:], AF.Relu, bias=t_ab1[:, j : j + 1])
            rh2_list.append(rh2)

        ps_a = ps_pool.tile([1, PS], f32, tag="s", bufs=1)
        for j in range(KC):
            nc.tensor.matmul(
                ps_a[:], t_aw2[:, j : j + 1], rh2_list[j][:],
                start=(j == 0), stop=(j == KC - 1),
            )

        negm = spool.tile([1, 1], f32, tag="negm")
        nc.vector.reduce_max(negm[:], ps_a[:], axis=AX.X, negate=True)
        ea = spool.tile([1, PS], f32, tag="ea")
        s_t = spool.tile([1, 1], f32, tag="s1")
        nc.scalar.activation(ea[:], ps_a[:], AF.Exp, bias=negm[:], accum_out=s_t[:])

        ebc = spool.tile([128, PS], f32, tag="ebc")
        nc.gpsimd.partition_broadcast(ebc[:], ea[:])

        part = spool.tile([128, KC], f32, tag="part")
        for k in range(KC):
            scr = spool.tile([128, PS], f32, tag="scr", name=f"scr_{k}")
            nc.vector.tensor_mul(scr[:], pfT[:, k, :].bitcast(f32), ebc[:])
            nc.vector.reduce_sum(part[:, k : k + 1], scr[:], axis=AX.X)
        nc.sync.dma_start(out_part.ap(), part[:])
        nc.sync.dma_start(out_stats.ap()[:, 0:1], negm[:])
        nc.sync.dma_start(out_stats.ap()[:, 1:2], s_t[:])

    nc.compile()
    return nc


def _get_program(blocks, npos, has_b1):
    key = (blocks, npos, has_b1)
    if key not in _PROG_CACHE:
        _PROG_CACHE[key] = _build_program(blocks, npos, has_b1)
    return _PROG_CACHE[key]


def _prep(inputs):
    """Host-side sharding/sorting/packing. Returns (blocks, npos, has_b1, in_maps)."""
    x = np.asarray(inputs["paths_nodes"], dtype=np.float32)
    lengths = np.asarray(inputs["lengths"], dtype=np.int32)
    pW1 = np.asarray(inputs["pW1"], dtype=np.float32)
    pb1 = np.asarray(inputs["pb1"], dtype=np.float32)
    pw2 = np.asarray(inputs["pw2"], dtype=np.float32)
    aW1 = np.asarray(inputs["aW1"], dtype=np.float32)
    ab1 = np.asarray(inputs["ab1"], dtype=np.float32)
    aw2 = np.asarray(inputs["aw2"], dtype=np.float32)
    # pb2 / ab2 shift their softmax logits uniformly -> no effect on output.

    bf = ml_dtypes.bfloat16
    # Deal paths round-robin by global sorted rank: core c gets ranks c, c+8, ...
    order_g = np.argsort(-lengths, kind="stable")          # [P] desc
    orders = order_g.reshape(PS, NCORES).T                 # [NC, PS]
    sorted_len = lengths[orders]                           # [NC, PS] desc per core
    len_max = sorted_len.max(axis=0)                       # [PS]
    blocks = _make_blocks(len_max)

    # Fold w2 into W1 columns: reorder by sign(w2), scale by |w2|.
    # relu(h) @ w2 == sum_pos relu(h*|w2|) - sum_neg relu(h*|w2|).
    pos = pw2 >= 0.0
    perm = np.argsort(~pos, kind="stable")                 # positives first
    npos = int(pos.sum())
    w1s = pW1[:, perm] * np.abs(pw2[perm])[None, :]
    b1s = pb1[perm] * np.abs(pw2[perm])
    has_b1 = bool(np.any(pb1 != 0.0))

    w1_np = np.ascontiguousarray(w1s.reshape(KC, 128, H)).astype(bf)
    b1r_np = b1s.reshape(1, H).astype(bf)
    aw1_np = np.ascontiguousarray(aW1.reshape(KC, 128, H)).astype(np.float32)
    ab1_np = np.ascontiguousarray(ab1.reshape(KC, 128).T).astype(np.float32)
    aw2_np = np.ascontiguousarray(aw2.reshape(KC, 128).T).astype(np.float32)
    one1 = np.ones((1, 128), dtype=bf)

    # score of an all-zero (padded) node row: relu(b1) @ w2  (pb2 dropped)
    c0 = float(np.maximum(pb1, 0.0) @ pw2)
    ec0 = float(np.exp(c0))

    ar = np.arange(LMAX + 4)
    in_maps = []
    for c in range(NCORES):
        xc = x[orders[c]]                             # [PS, LMAX, D] sorted
        lc = sorted_len[c]                            # [PS]
        xr_parts = []
        npad_vals = np.empty(PS, dtype=np.float32)
        p = 0
        for (bp, cap) in blocks:
            lb = lc[p : p + bp]
            ccap = min(cap, LMAX)
            xblk = xc[p : p + bp, :ccap, :]           # [bp, ccap, D]
            mask = ar[None, :ccap, None] < lb[:, None, None]
            xblk = np.where(mask, xblk, 0.0).astype(bf)
            if ccap < cap:                            # mult-4 pad slot(s)
                pad = np.zeros((bp, cap - ccap, D), dtype=bf)
                xblk = np.concatenate([xblk, pad], axis=1)
            xb_t = (
                xblk.reshape(bp, cap, KC, 128)
                .transpose(2, 3, 0, 1)
                .reshape(KC, 128, bp * cap)
            )
            xr_parts.append(xb_t.ravel())
            npad_vals[p : p + bp] = (cap - lb).astype(np.float32) * ec0
            p += bp
        npad_np = np.broadcast_to(npad_vals, (128, PS)).copy()
        in_maps.append(
            {
                "xb": np.concatenate(xr_parts),
                "npad": npad_np,
                "w1": w1_np,
                "b1r": b1r_np,
                "aw1": aw1_np,
                "ab1": ab1_np,
                "aw2": aw2_np,
                "one1_bf": one1,
            }
        )
    return blocks, npos, has_b1, in_maps


def kernel(**inputs):
    global LAST_RESULT
    blocks, npos, has_b1, in_maps = _prep(inputs)
    nc = _get_program(blocks, npos, has_b1)

    res = bass_utils.run_bass_kernel_spmd(
        nc, in_maps, core_ids=list(range(NCORES)), **_TRACE_KW
    )
    LAST_RESULT = res

    parts = np.stack([r["out_part"] for r in res.results])    # [8, 128, KC]
    stats = np.stack([r["out_stats"] for r in res.results])   # [8, 1, 2]
    m = -stats[:, 0, 0]
    s = stats[:, 0, 1]
    mg = m.max()
    sc = np.exp(m - mg)
    total = float((sc * s).sum())
    vec = (parts * sc[:, None, None]).sum(axis=0)             # [128, KC]
    user = np.ascontiguousarray(vec.T).reshape(D) / total
    return user.astype(np.float32)
